# revision 14
# baseline (speedup 1.0000x reference)
"""CosSim2D (3x3, same-pad) Trainium2 kernel, 8-core batch-parallel.

Layout strategy per core (one 224x224x32 image):
  - Host pads the image to 226x226, flattens channel-major, and packs 4
    overlapping quarter strips (12992 px + 454 halo each) onto the 4
    partition groups: xpb[32g+c, j] = x[c, g*12992 + j], bf16.
  - Device: per 448-px round, 9 accumulating matmuls (one per 3x3 tap)
    with a [128, 128] block-diagonal stationary -- 4 identical [32c, 32f]
    normalized-weight blocks on the diagonal -- so all 4 partition groups'
    convolutions ride a single K=128 column stream.  Tap shifts are free-
    dim offsets (dy*226 + dx) on the rhs AP.
  - Evac: scalar-engine Copy casts PSUM f32 -> SBUF bf16; DMA to HBM in
    [round, 32g+f, col] blocks.
  - Host: unblocks, computes the x-side norm (3x3 box sum of per-pixel
    squared sums -- input-only, no device data needed), divides, applies
    sign*(|x|+eps)^e, casts to f32.
"""

import numpy as np

import concourse.bass as bass
import concourse.mybir as mybir
import concourse.tile as tile
from concourse import bacc
from concourse.bass_utils import run_bass_kernel_spmd

K = 3
EPS = 1e-12
H = W = 224
C = 32
F = 32
B = 8
XP = 226                  # padded row stride

CH = 512                  # px per round (= matmul N, one full PSUM bank)
ROUNDS = 25               # per partition group
Q = ROUNDS * CH           # 12800 px per group (4*Q = 51200 >= 50622 used)
HALO = 2 * XP + 2         # max tap offset (dy=2, dx=2)
COLS = Q + HALO           # 13254 columns per packed strip row
BANDS = (1, 1, 2, 4, 6, 6, 5)  # rounds per band: ramp so matmuls start early

_compiled = None
TRACE = False
LAST_PROFILE = None


def _build():
    nc = bacc.Bacc()
    f32 = mybir.dt.float32
    bf16 = mybir.dt.bfloat16

    xpb = nc.declare_dram_parameter("xpb", [128 * COLS], bf16, isOutput=False)
    wtd = nc.declare_dram_parameter("wtd", [128 * 9 * 128], bf16, isOutput=False)
    odev = nc.declare_dram_parameter("odev", [128, ROUNDS * CH], bf16, isOutput=True)

    with tile.TileContext(nc) as tc:
        with (
            tc.tile_pool(name="consts", bufs=1) as consts,
            tc.tile_pool(name="band", bufs=3) as band_pool,
            tc.tile_pool(name="out", bufs=4) as out_pool,
            tc.tile_pool(name="psum", bufs=4, space="PSUM") as psum_pool,
        ):
            xp2d = xpb.rearrange("(p j) -> p j", j=COLS)
            wt2d = wtd.rearrange("(p x) -> p x", x=9 * 128)
            b0w = BANDS[0] * CH + HALO
            T0 = band_pool.tile([128, b0w], bf16, tag="T")
            nc.sync.dma_start(out=T0, in_=xp2d[:, 0:b0w])
            wts = consts.tile([128, 9 * 128], bf16, tag="wts")
            nc.sync.dma_start(out=wts, in_=wt2d)

            rr = 0
            for bi, nr in enumerate(BANDS):
                if bi == 0:
                    T = T0
                else:
                    b0 = rr * CH
                    bw = nr * CH + HALO
                    T = band_pool.tile([128, bw], bf16, tag="T")
                    nc.sync.dma_start(out=T, in_=xp2d[:, b0 : b0 + bw])
                for r in range(nr):
                    P = psum_pool.tile([128, CH], f32, tag="P")
                    for t in range(9):
                        off = r * CH + (t // 3) * XP + (t % 3)
                        nc.tensor.matmul(
                            P,
                            wts[:, t * 128 : (t + 1) * 128],
                            T[:, off : off + CH],
                            start=(t == 0),
                            stop=(t == 8),
                        )
                    O = out_pool.tile([128, CH], bf16, tag="O")
                    nc.vector.tensor_copy(O, P)
                    dst = odev[:, rr * CH : (rr + 1) * CH]
                    if rr >= ROUNDS - 2:
                        for qq in range(4):
                            nc.sync.dma_start(
                                out=dst[qq * 32 : (qq + 1) * 32],
                                in_=O[qq * 32 : (qq + 1) * 32, :],
                            )
                    else:
                        nc.sync.dma_start(out=dst, in_=O)
                    rr += 1

    nc.compile()
    return nc


def _host_pack_image(image_b):
    """One core's input: padded channel-major image in 4 overlapping strips."""
    import ml_dtypes

    padded = np.zeros((XP, XP, C), dtype=np.float32)
    padded[1:225, 1:225, :] = image_b
    flat = padded.transpose(2, 0, 1).reshape(C, XP * XP)
    pc = np.zeros((C, 4 * Q + HALO), dtype=np.float32)
    pc[:, : XP * XP] = flat
    xpb = np.empty((4, C, COLS), dtype=np.float32)
    for g in range(4):
        xpb[g] = pc[:, g * Q : g * Q + COLS]
    return xpb.reshape(128 * COLS).astype(ml_dtypes.bfloat16)


def _host_pack_weights(w, q):
    """Block-diagonal normalized-weight stationaries, [128, 9, 128] bf16."""
    import ml_dtypes

    qtv = np.float32(np.float32(q[0]) * np.float32(q[0]) / np.float32(10.0))
    w0 = w[0].astype(np.float32)  # [288, 32], row = (dy*3+dx)*C + c
    wn = np.sqrt(np.maximum((w0 * w0).sum(axis=0), np.float32(EPS))) + qtv
    wn9 = (w0 / wn[None, :]).reshape(9, C, F)
    wtbd = np.zeros((128, 9, 128), dtype=np.float32)
    for g in range(4):
        wtbd[32 * g : 32 * g + 32, :, 32 * g : 32 * g + 32] = wn9.transpose(1, 0, 2)
    return wtbd.reshape(-1).astype(ml_dtypes.bfloat16), float(qtv)


_PMAP = None


def _pmap():
    global _PMAP
    if _PMAP is None:
        y, x = np.mgrid[0:H, 0:W]
        _PMAP = (y * XP + x).reshape(-1)
    return _PMAP


def kernel(image, w, p, q):
    global _compiled
    image = np.asarray(image, dtype=np.float32)
    w = np.asarray(w, dtype=np.float32)
    p = np.asarray(p, dtype=np.float32)
    q = np.asarray(q, dtype=np.float32)

    wtd, qtv = _host_pack_weights(w, q)
    in_maps = [
        {"xpb": _host_pack_image(image[b]), "wtd": wtd} for b in range(B)
    ]

    if _compiled is None:
        _compiled = _build()
    nc = _compiled

    global LAST_PROFILE
    res = run_bass_kernel_spmd(
        nc, in_maps, core_ids=list(range(B)), trace=TRACE
    )
    LAST_PROFILE = res
    if TRACE and res.exec_time_ns is not None:
        print(f"HW exec time: {res.exec_time_ns} ns")

    # x-side norm: 3x3 same-pad box sum of per-pixel squared channel sums.
    s2 = np.square(image).sum(axis=3)  # [B, 224, 224]
    sp = np.zeros((B, XP, XP), dtype=np.float32)
    sp[:, 1:225, 1:225] = s2
    ns = np.zeros((B, H, W), dtype=np.float32)
    for dy in range(K):
        for dx in range(K):
            ns += sp[:, dy : dy + H, dx : dx + W]
    xn = np.sqrt(np.maximum(ns, np.float32(EPS))) + qtv  # [B, 224, 224]

    e = (p * p) / np.float32(100.0)  # per-filter exponent
    pm = _pmap()
    out = np.empty((B, H * W, F), dtype=np.float32)
    for b in range(B):
        arr = np.asarray(res.results[b]["odev"], dtype=np.float32)
        conv_p = arr.reshape(4, 32, ROUNDS * CH).transpose(0, 2, 1)
        conv_p = conv_p.reshape(4 * Q, F)[pm]  # [H*W, F]
        sim = conv_p / xn[b].reshape(-1)[:, None]
        out[b] = np.sign(sim) * np.power(np.abs(sim) + np.float32(EPS), e[None, :])
    return out.reshape(B, H, W, F)


# revision 15
# speedup vs baseline: 1.0247x; 1.0247x over previous
"""CosSim2D (3x3, same-pad) Trainium2 kernel, 8-core batch-parallel.

Layout strategy per core (one 224x224x32 image):
  - Host pads the image to 226x226, flattens channel-major, and packs 4
    overlapping quarter strips (12992 px + 454 halo each) onto the 4
    partition groups: xpb[32g+c, j] = x[c, g*12992 + j], bf16.
  - Device: per 448-px round, 9 accumulating matmuls (one per 3x3 tap)
    with a [128, 128] block-diagonal stationary -- 4 identical [32c, 32f]
    normalized-weight blocks on the diagonal -- so all 4 partition groups'
    convolutions ride a single K=128 column stream.  Tap shifts are free-
    dim offsets (dy*226 + dx) on the rhs AP.
  - Evac: scalar-engine Copy casts PSUM f32 -> SBUF bf16; DMA to HBM in
    [round, 32g+f, col] blocks.
  - Host: unblocks, computes the x-side norm (3x3 box sum of per-pixel
    squared sums -- input-only, no device data needed), divides, applies
    sign*(|x|+eps)^e, casts to f32.
"""

import numpy as np

import concourse.bass as bass
import concourse.mybir as mybir
import concourse.tile as tile
from concourse import bacc
from concourse.bass_utils import run_bass_kernel_spmd

K = 3
EPS = 1e-12
H = W = 224
C = 32
F = 32
B = 8
XP = 226                  # padded row stride

CH = 512                  # px per round (= matmul N, one full PSUM bank)
ROUNDS = 25               # per partition group
Q = ROUNDS * CH           # 12800 px per group (4*Q = 51200 >= 50622 used)
HALO = 2 * XP + 2         # max tap offset (dy=2, dx=2)
COLS = Q + HALO           # 13254 columns per packed strip row
BANDS = (1, 1, 2, 4, 6, 6, 5)  # rounds per band: ramp so matmuls start early

_compiled = None
TRACE = False
LAST_PROFILE = None


def _build():
    nc = bacc.Bacc()
    f32 = mybir.dt.float32
    bf16 = mybir.dt.bfloat16

    xpb = nc.declare_dram_parameter("xpb", [128 * COLS], bf16, isOutput=False)
    wtd = nc.declare_dram_parameter("wtd", [128 * 9 * 128], bf16, isOutput=False)
    odev = nc.declare_dram_parameter("odev", [128, ROUNDS * CH], bf16, isOutput=True)

    with tile.TileContext(nc) as tc:
        with (
            tc.tile_pool(name="consts", bufs=1) as consts,
            tc.tile_pool(name="band", bufs=3) as band_pool,
            tc.tile_pool(name="out", bufs=4) as out_pool,
            tc.tile_pool(name="psum", bufs=4, space="PSUM") as psum_pool,
        ):
            xp2d = xpb.rearrange("(p j) -> p j", j=COLS)
            wt2d = wtd.rearrange("(p x) -> p x", x=9 * 128)
            b0w = BANDS[0] * CH + HALO
            T0 = band_pool.tile([128, b0w], bf16, tag="T")
            nc.sync.dma_start(out=T0, in_=xp2d[:, 0:b0w])
            wts = consts.tile([128, 9 * 128], bf16, tag="wts")
            nc.sync.dma_start(out=wts, in_=wt2d)

            rr = 0
            for bi, nr in enumerate(BANDS):
                if bi == 0:
                    T = T0
                else:
                    b0 = rr * CH
                    bw = nr * CH + HALO
                    T = band_pool.tile([128, bw], bf16, tag="T")
                    nc.sync.dma_start(out=T, in_=xp2d[:, b0 : b0 + bw])
                for r in range(nr):
                    P = psum_pool.tile([128, CH], f32, tag="P")
                    for t in range(9):
                        off = r * CH + (t // 3) * XP + (t % 3)
                        nc.tensor.matmul(
                            P,
                            wts[:, t * 128 : (t + 1) * 128],
                            T[:, off : off + CH],
                            start=(t == 0),
                            stop=(t == 8),
                        )
                    O = out_pool.tile([128, CH], bf16, tag="O")
                    nc.vector.tensor_copy(O, P)
                    dst = odev[:, rr * CH : (rr + 1) * CH]
                    if rr >= ROUNDS - 2:
                        for qq in range(4):
                            nc.sync.dma_start(
                                out=dst[qq * 32 : (qq + 1) * 32],
                                in_=O[qq * 32 : (qq + 1) * 32, :],
                            )
                    else:
                        nc.sync.dma_start(out=dst[0:64], in_=O[0:64, :])
                        nc.sync.dma_start(out=dst[64:128], in_=O[64:128, :])
                    rr += 1

    nc.compile()
    return nc


def _host_pack_image(image_b):
    """One core's input: padded channel-major image in 4 overlapping strips."""
    import ml_dtypes

    padded = np.zeros((XP, XP, C), dtype=np.float32)
    padded[1:225, 1:225, :] = image_b
    flat = padded.transpose(2, 0, 1).reshape(C, XP * XP)
    pc = np.zeros((C, 4 * Q + HALO), dtype=np.float32)
    pc[:, : XP * XP] = flat
    xpb = np.empty((4, C, COLS), dtype=np.float32)
    for g in range(4):
        xpb[g] = pc[:, g * Q : g * Q + COLS]
    return xpb.reshape(128 * COLS).astype(ml_dtypes.bfloat16)


def _host_pack_weights(w, q):
    """Block-diagonal normalized-weight stationaries, [128, 9, 128] bf16."""
    import ml_dtypes

    qtv = np.float32(np.float32(q[0]) * np.float32(q[0]) / np.float32(10.0))
    w0 = w[0].astype(np.float32)  # [288, 32], row = (dy*3+dx)*C + c
    wn = np.sqrt(np.maximum((w0 * w0).sum(axis=0), np.float32(EPS))) + qtv
    wn9 = (w0 / wn[None, :]).reshape(9, C, F)
    wtbd = np.zeros((128, 9, 128), dtype=np.float32)
    for g in range(4):
        wtbd[32 * g : 32 * g + 32, :, 32 * g : 32 * g + 32] = wn9.transpose(1, 0, 2)
    return wtbd.reshape(-1).astype(ml_dtypes.bfloat16), float(qtv)


_PMAP = None


def _pmap():
    global _PMAP
    if _PMAP is None:
        y, x = np.mgrid[0:H, 0:W]
        _PMAP = (y * XP + x).reshape(-1)
    return _PMAP


def kernel(image, w, p, q):
    global _compiled
    image = np.asarray(image, dtype=np.float32)
    w = np.asarray(w, dtype=np.float32)
    p = np.asarray(p, dtype=np.float32)
    q = np.asarray(q, dtype=np.float32)

    wtd, qtv = _host_pack_weights(w, q)
    in_maps = [
        {"xpb": _host_pack_image(image[b]), "wtd": wtd} for b in range(B)
    ]

    if _compiled is None:
        _compiled = _build()
    nc = _compiled

    global LAST_PROFILE
    res = run_bass_kernel_spmd(
        nc, in_maps, core_ids=list(range(B)), trace=TRACE
    )
    LAST_PROFILE = res
    if TRACE and res.exec_time_ns is not None:
        print(f"HW exec time: {res.exec_time_ns} ns")

    # x-side norm: 3x3 same-pad box sum of per-pixel squared channel sums.
    s2 = np.square(image).sum(axis=3)  # [B, 224, 224]
    sp = np.zeros((B, XP, XP), dtype=np.float32)
    sp[:, 1:225, 1:225] = s2
    ns = np.zeros((B, H, W), dtype=np.float32)
    for dy in range(K):
        for dx in range(K):
            ns += sp[:, dy : dy + H, dx : dx + W]
    xn = np.sqrt(np.maximum(ns, np.float32(EPS))) + qtv  # [B, 224, 224]

    e = (p * p) / np.float32(100.0)  # per-filter exponent
    pm = _pmap()
    out = np.empty((B, H * W, F), dtype=np.float32)
    for b in range(B):
        arr = np.asarray(res.results[b]["odev"], dtype=np.float32)
        conv_p = arr.reshape(4, 32, ROUNDS * CH).transpose(0, 2, 1)
        conv_p = conv_p.reshape(4 * Q, F)[pm]  # [H*W, F]
        sim = conv_p / xn[b].reshape(-1)[:, None]
        out[b] = np.sign(sim) * np.power(np.abs(sim) + np.float32(EPS), e[None, :])
    return out.reshape(B, H, W, F)


# revision 16
# speedup vs baseline: 1.0552x; 1.0297x over previous
"""CosSim2D (3x3, same-pad) Trainium2 kernel, 8-core batch-parallel.

Layout strategy per core (one 224x224x32 image):
  - Host pads the image to 226x226, flattens channel-major, and packs 4
    overlapping quarter strips (12992 px + 454 halo each) onto the 4
    partition groups: xpb[32g+c, j] = x[c, g*12992 + j], bf16.
  - Device: per 448-px round, 9 accumulating matmuls (one per 3x3 tap)
    with a [128, 128] block-diagonal stationary -- 4 identical [32c, 32f]
    normalized-weight blocks on the diagonal -- so all 4 partition groups'
    convolutions ride a single K=128 column stream.  Tap shifts are free-
    dim offsets (dy*226 + dx) on the rhs AP.
  - Evac: scalar-engine Copy casts PSUM f32 -> SBUF bf16; DMA to HBM in
    [round, 32g+f, col] blocks.
  - Host: unblocks, computes the x-side norm (3x3 box sum of per-pixel
    squared sums -- input-only, no device data needed), divides, applies
    sign*(|x|+eps)^e, casts to f32.
"""

import numpy as np

import concourse.bass as bass
import concourse.mybir as mybir
import concourse.tile as tile
from concourse import bacc
from concourse.bass_utils import run_bass_kernel_spmd

K = 3
EPS = 1e-12
H = W = 224
C = 32
F = 32
B = 8
XP = 226                  # padded row stride

CH = 512                  # px per round (= matmul N, one full PSUM bank)
ROUNDS = 25               # per partition group
Q = ROUNDS * CH           # 12800 px per group (4*Q = 51200 >= 50622 used)
HALO = 2 * XP + 2         # max tap offset (dy=2, dx=2)
COLS = Q + HALO           # 13254 columns per packed strip row
BANDS = (1, 1, 2, 4, 6, 6, 5)  # rounds per band: ramp so matmuls start early

_compiled = None
TRACE = False
LAST_PROFILE = None


def _build():
    nc = bacc.Bacc()
    f32 = mybir.dt.float32
    bf16 = mybir.dt.bfloat16

    xpb = nc.declare_dram_parameter("xpb", [128 * COLS], bf16, isOutput=False)
    wtd = nc.declare_dram_parameter("wtd", [128 * 9 * 128], bf16, isOutput=False)
    odev = nc.declare_dram_parameter("odev", [128, ROUNDS * CH], bf16, isOutput=True)

    with tile.TileContext(nc) as tc:
        with (
            tc.tile_pool(name="consts", bufs=1) as consts,
            tc.tile_pool(name="band", bufs=3) as band_pool,
            tc.tile_pool(name="out", bufs=4) as out_pool,
            tc.tile_pool(name="psum", bufs=4, space="PSUM") as psum_pool,
        ):
            xp2d = xpb.rearrange("(p j) -> p j", j=COLS)
            wt2d = wtd.rearrange("(p x) -> p x", x=9 * 128)
            b0w = BANDS[0] * CH + HALO
            T0 = band_pool.tile([128, b0w], bf16, tag="T")
            nc.sync.dma_start(out=T0, in_=xp2d[:, 0:b0w])
            wts = consts.tile([128, 9 * 128], bf16, tag="wts")
            nc.sync.dma_start(out=wts, in_=wt2d)

            rr = 0
            for bi, nr in enumerate(BANDS):
                if bi == 0:
                    T = T0
                else:
                    b0 = rr * CH
                    bw = nr * CH + HALO
                    T = band_pool.tile([128, bw], bf16, tag="T")
                    nc.sync.dma_start(out=T, in_=xp2d[:, b0 : b0 + bw])
                for r in range(nr):
                    P = psum_pool.tile([128, CH], f32, tag="P")
                    for t in range(9):
                        off = r * CH + (t // 3) * XP + (t % 3)
                        nc.tensor.matmul(
                            P,
                            wts[:, t * 128 : (t + 1) * 128],
                            T[:, off : off + CH],
                            start=(t == 0),
                            stop=(t == 8),
                        )
                    O = out_pool.tile([128, CH], bf16, tag="O")
                    nc.vector.tensor_copy(O, P)
                    dst = odev[:, rr * CH : (rr + 1) * CH]
                    nc.sync.dma_start(out=dst[0:64], in_=O[0:64, :])
                    nc.sync.dma_start(out=dst[64:128], in_=O[64:128, :])
                    rr += 1

    nc.compile()
    return nc


def _host_pack_image(image_b):
    """One core's input: padded channel-major image in 4 overlapping strips."""
    import ml_dtypes

    padded = np.zeros((XP, XP, C), dtype=np.float32)
    padded[1:225, 1:225, :] = image_b
    flat = padded.transpose(2, 0, 1).reshape(C, XP * XP)
    pc = np.zeros((C, 4 * Q + HALO), dtype=np.float32)
    pc[:, : XP * XP] = flat
    xpb = np.empty((4, C, COLS), dtype=np.float32)
    for g in range(4):
        xpb[g] = pc[:, g * Q : g * Q + COLS]
    return xpb.reshape(128 * COLS).astype(ml_dtypes.bfloat16)


def _host_pack_weights(w, q):
    """Block-diagonal normalized-weight stationaries, [128, 9, 128] bf16."""
    import ml_dtypes

    qtv = np.float32(np.float32(q[0]) * np.float32(q[0]) / np.float32(10.0))
    w0 = w[0].astype(np.float32)  # [288, 32], row = (dy*3+dx)*C + c
    wn = np.sqrt(np.maximum((w0 * w0).sum(axis=0), np.float32(EPS))) + qtv
    wn9 = (w0 / wn[None, :]).reshape(9, C, F)
    wtbd = np.zeros((128, 9, 128), dtype=np.float32)
    for g in range(4):
        wtbd[32 * g : 32 * g + 32, :, 32 * g : 32 * g + 32] = wn9.transpose(1, 0, 2)
    return wtbd.reshape(-1).astype(ml_dtypes.bfloat16), float(qtv)


_PMAP = None


def _pmap():
    global _PMAP
    if _PMAP is None:
        y, x = np.mgrid[0:H, 0:W]
        _PMAP = (y * XP + x).reshape(-1)
    return _PMAP


def kernel(image, w, p, q):
    global _compiled
    image = np.asarray(image, dtype=np.float32)
    w = np.asarray(w, dtype=np.float32)
    p = np.asarray(p, dtype=np.float32)
    q = np.asarray(q, dtype=np.float32)

    wtd, qtv = _host_pack_weights(w, q)
    in_maps = [
        {"xpb": _host_pack_image(image[b]), "wtd": wtd} for b in range(B)
    ]

    if _compiled is None:
        _compiled = _build()
    nc = _compiled

    global LAST_PROFILE
    res = run_bass_kernel_spmd(
        nc, in_maps, core_ids=list(range(B)), trace=TRACE
    )
    LAST_PROFILE = res
    if TRACE and res.exec_time_ns is not None:
        print(f"HW exec time: {res.exec_time_ns} ns")

    # x-side norm: 3x3 same-pad box sum of per-pixel squared channel sums.
    s2 = np.square(image).sum(axis=3)  # [B, 224, 224]
    sp = np.zeros((B, XP, XP), dtype=np.float32)
    sp[:, 1:225, 1:225] = s2
    ns = np.zeros((B, H, W), dtype=np.float32)
    for dy in range(K):
        for dx in range(K):
            ns += sp[:, dy : dy + H, dx : dx + W]
    xn = np.sqrt(np.maximum(ns, np.float32(EPS))) + qtv  # [B, 224, 224]

    e = (p * p) / np.float32(100.0)  # per-filter exponent
    pm = _pmap()
    out = np.empty((B, H * W, F), dtype=np.float32)
    for b in range(B):
        arr = np.asarray(res.results[b]["odev"], dtype=np.float32)
        conv_p = arr.reshape(4, 32, ROUNDS * CH).transpose(0, 2, 1)
        conv_p = conv_p.reshape(4 * Q, F)[pm]  # [H*W, F]
        sim = conv_p / xn[b].reshape(-1)[:, None]
        out[b] = np.sign(sim) * np.power(np.abs(sim) + np.float32(EPS), e[None, :])
    return out.reshape(B, H, W, F)
